# revision 4
# baseline (speedup 1.0000x reference)
"""Trainium2 Bass kernel for DictionaryExpertLISTA.

Model: 5 LISTA layers of u = x@W[l] + z@S[l]; z = topk_mask(u, k=32 by |u|);
final recon = z @ D.T.  Returns (recon, z).

Sharding: data-parallel over batch across 8 NeuronCores (x, z sharded on
dim 0; W/S/D replicated).  No cross-core communication.

Per-core structure (B_loc=4096 rows = 32 tiles of 128):
  layer-outer loop, tile-inner.  S_l (16 MiB) + W_l (4 MiB) resident in SBUF
  per layer.  u accumulated in PSUM via fp32 matmuls (lhsT = host-transposed
  x / producer-transposed z chunks).  Top-k per row via DVE max8 +
  match_replace(-1e30) x4 rounds, then is_equal mask + multiply.  z is
  PE-transposed per tile and round-trips DRAM in transposed layout for the
  next layer's contraction.
"""
import sys
for p in ("/opt/trn_rl_repo", "/root/.axon_site/_ro/trn_rl_repo"):
    if p not in sys.path:
        sys.path.insert(0, p)

import numpy as np

N_CORES = 8
B = 32768
INPUT = 512
CODE = 2048
L = 5
K = 32
P = 128
B_LOC = B // N_CORES           # 4096
TILES = B_LOC // P             # 32
KC = INPUT // P                # 4 contract chunks for x
CC = CODE // P                 # 16 contract chunks for z
NOUT = CODE // 512             # 4 psum out chunks of 512
NEG = -1.0e30

_CACHE = {}


def _build():
    import concourse.bacc as bacc
    import concourse.mybir as mybir
    import concourse.tile as tile

    F32 = mybir.dt.float32
    ACTF = mybir.ActivationFunctionType
    ALU = mybir.AluOpType

    nc = bacc.Bacc(None, target_bir_lowering=False)

    xT = nc.declare_dram_parameter("xT", [INPUT, B_LOC], F32, isOutput=False)
    Wd = nc.declare_dram_parameter("W", [L, INPUT, CODE], F32, isOutput=False)
    Sd = nc.declare_dram_parameter("S", [L, CODE, CODE], F32, isOutput=False)
    DT = nc.declare_dram_parameter("DT", [CODE, INPUT], F32, isOutput=False)
    ident = nc.declare_dram_parameter("ident", [P, P], F32, isOutput=False)
    recon = nc.declare_dram_parameter("recon", [B_LOC, INPUT], F32, isOutput=True)
    z_out = nc.declare_dram_parameter("z_out", [B_LOC, CODE], F32, isOutput=True)

    zT_dram = nc.dram_tensor("zT_scratch", [TILES, P, CODE], F32)

    with tile.TileContext(nc) as tc:
        with tc.tile_pool(name="wpool", bufs=1) as wp, \
             tc.tile_pool(name="work", bufs=2) as wk, \
             tc.tile_pool(name="usb", bufs=1) as up, \
             tc.tile_pool(name="small", bufs=8) as sp, \
             tc.tile_pool(name="psum", bufs=2, space="PSUM") as ps:

            id_sb = wp.tile([P, P], F32, tag="ident")
            nc.sync.dma_start(out=id_sb[:], in_=ident[:])

            for l in range(L):
                # resident weights for this layer
                w_sb = wp.tile([P, KC, CODE], F32, tag="W")
                for kc in range(KC):
                    nc.sync.dma_start(
                        out=w_sb[:, kc, :],
                        in_=Wd[l, kc * P:(kc + 1) * P, :])
                if l > 0:
                    s_sb = wp.tile([P, CC, CODE], F32, tag="S")
                    for c in range(CC):
                        nc.sync.dma_start(
                            out=s_sb[:, c, :],
                            in_=Sd[l, c * P:(c + 1) * P, :])

                for t in range(TILES):
                    # inputs for this tile
                    xt = wk.tile([P, KC, P], F32, tag="xT")
                    nc.sync.dma_start(
                        out=xt[:],
                        in_=xT[:, t * P:(t + 1) * P].rearrange(
                            "(kc p) b -> p kc b", p=P))
                    if l > 0:
                        zt_in = wk.tile([P, CC, P], F32, tag="zT")
                        nc.scalar.dma_start(
                            out=zt_in[:],
                            in_=zT_dram[t].rearrange("p (cc b) -> p cc b", b=P))

                    # u = x @ W_l (+ z @ S_l)
                    u = ps.tile([P, CODE], F32, tag="ps")
                    for n in range(NOUT):
                        ncontract = KC + (CC if l > 0 else 0)
                        ci = 0
                        for kc in range(KC):
                            nc.tensor.matmul(
                                u[:, n * 512:(n + 1) * 512],
                                xt[:, kc, :],
                                w_sb[:, kc, n * 512:(n + 1) * 512],
                                start=(ci == 0), stop=(ci == ncontract - 1))
                            ci += 1
                        if l > 0:
                            for c in range(CC):
                                nc.tensor.matmul(
                                    u[:, n * 512:(n + 1) * 512],
                                    zt_in[:, c, :],
                                    s_sb[:, c, n * 512:(n + 1) * 512],
                                    start=(ci == 0), stop=(ci == ncontract - 1))
                                ci += 1

                    # abs + copy out of PSUM (frees u's banks fast)
                    a = wk.tile([P, CODE], F32, tag="a")
                    nc.scalar.activation(a[:], u[:], ACTF.Abs)
                    uz = up.tile([P, CODE], F32, tag="u")
                    nc.scalar.activation(uz[:], u[:], ACTF.Copy)

                    # top-32 by |u|: 4 rounds max8 + match_replace
                    for r in range(4):
                        m8 = sp.tile([P, 8], F32, tag="m8")
                        nc.vector.max(m8[:], a[:])
                        nc.vector.match_replace(a[:], m8[:], a[:], NEG)
                    # mask = (a == NEG); z = u * mask   (z overwrites uz)
                    nc.vector.tensor_scalar(a[:], a[:], NEG, None, ALU.is_equal)
                    nc.vector.tensor_tensor(uz[:], uz[:], a[:], ALU.mult)

                    # transpose z for next layer's contraction
                    zt_ps = ps.tile([P, CODE], F32, tag="ps")
                    for c in range(CC):
                        nc.tensor.transpose(
                            zt_ps[:, c * P:(c + 1) * P],
                            uz[:, c * P:(c + 1) * P],
                            id_sb[:])
                    zt_sb = wk.tile([P, CODE], F32, tag="a")
                    nc.scalar.activation(zt_sb[:], zt_ps[:], ACTF.Copy)
                    nc.gpsimd.dma_start(out=zT_dram[t], in_=zt_sb[:])

                    if l == L - 1:
                        nc.sync.dma_start(
                            out=z_out[t * P:(t + 1) * P, :], in_=uz[:])

            # recon = z @ D.T
            dt_sb = wp.tile([P, CC, INPUT], F32, tag="S")
            for c in range(CC):
                nc.sync.dma_start(
                    out=dt_sb[:, c, :], in_=DT[c * P:(c + 1) * P, :])
            for t in range(TILES):
                zt_in = wk.tile([P, CC, P], F32, tag="zT")
                nc.scalar.dma_start(
                    out=zt_in[:],
                    in_=zT_dram[t].rearrange("p (cc b) -> p cc b", b=P))
                r_ps = ps.tile([P, INPUT], F32, tag="ps")
                for c in range(CC):
                    nc.tensor.matmul(
                        r_ps[:], zt_in[:, c, :], dt_sb[:, c, :],
                        start=(c == 0), stop=(c == CC - 1))
                r_sb = up.tile([P, INPUT], F32, tag="u")
                nc.scalar.activation(r_sb[:], r_ps[:], ACTF.Copy)
                nc.sync.dma_start(out=recon[t * P:(t + 1) * P, :], in_=r_sb[:])

    nc.compile()
    return nc


def get_nc():
    if "nc" not in _CACHE:
        _CACHE["nc"] = _build()
    return _CACHE["nc"]


def kernel(x, W, S, D):
    from concourse.bass_utils import run_bass_kernel_spmd

    nc = get_nc()
    x = np.ascontiguousarray(x, dtype=np.float32)
    W = np.ascontiguousarray(W, dtype=np.float32)
    S = np.ascontiguousarray(S, dtype=np.float32)
    D = np.ascontiguousarray(D, dtype=np.float32)

    xT = np.ascontiguousarray(x.T)                 # [INPUT, B]
    DTm = np.ascontiguousarray(D.T)                # [CODE, INPUT]
    ident = np.eye(P, dtype=np.float32)

    in_maps = []
    for c in range(N_CORES):
        in_maps.append(dict(
            xT=np.ascontiguousarray(xT[:, c * B_LOC:(c + 1) * B_LOC]),
            W=W, S=S, DT=DTm, ident=ident,
        ))
    res = run_bass_kernel_spmd(nc, in_maps, list(range(N_CORES)))
    recon = np.concatenate([r["recon"] for r in res.results], axis=0)
    z = np.concatenate([r["z_out"] for r in res.results], axis=0)
    return recon, z
